# revision 1
# baseline (speedup 1.0000x reference)
"""JPEGBase (nn_JPEGBase_240518169043) Trainium2 kernel.

The reference computes rgb->yuv, *255, blockwise 8x8 DCT, blockwise IDCT
(compress() is identity), /255, yuv->rgb.  The orthonormal DCT/IDCT pair and
the *255 / /255 cancel exactly, so the remaining math is a per-pixel 3x3
color-matrix roundtrip A = yuv2rgb @ rgb2yuv applied along the channel dim
(float32 discrepancy vs. the reference's explicit DCT roundtrip is ~1.5e-7
relative).  i_co is unused by the reference.

Sharding: pure data parallelism - batch 32 -> 4 images per core across 8
cores.  Per core the kernel streams 4 images of [3,512,512] f32 through SBUF
([128,2048] per plane), computes the three output planes as weighted sums of
the three input planes (2 DVE scalar_tensor_tensor ops + 1 ACT scale per
output plane), and streams them back.  Memory-bound: ~25 MB of HBM traffic
per core.
"""

import numpy as np
from contextlib import ExitStack

import concourse.bass as bass  # noqa: F401  (engine namespaces live on nc)
import concourse.tile as tile
from concourse import bacc, mybir
from concourse.bass_utils import run_bass_kernel_spmd

N_CORES = 8
B_FULL = 32
B_PER_CORE = B_FULL // N_CORES  # 4
C = 3
H = 512
W = 512
P = 128               # SBUF partitions
F = (H * W) // P      # 2048 floats per partition per plane


def _color_matrix():
    # kornia rgb_to_yuv / yuv_to_rgb coefficient matrices, composed in f64.
    m = np.array(
        [[0.299, 0.587, 0.114],
         [-0.147, -0.289, 0.436],
         [0.615, -0.515, -0.100]], dtype=np.float64)
    n = np.array(
        [[1.0, 0.0, 1.14],
         [1.0, -0.396, -0.581],
         [1.0, 2.029, 0.0]], dtype=np.float64)
    return n @ m


def build_nc():
    """Build + compile the per-core Bass program (same program on all cores)."""
    a = _color_matrix()
    nc = bacc.Bacc(
        "TRN2", target_bir_lowering=False, debug=False, num_devices=N_CORES
    )
    x = nc.dram_tensor(
        "x", [B_PER_CORE, C, H, W], mybir.dt.float32, kind="ExternalInput"
    ).ap()
    y = nc.dram_tensor(
        "y", [B_PER_CORE, C, H, W], mybir.dt.float32, kind="ExternalOutput"
    ).ap()
    # [b, c, 128, 2048]; partition p covers image rows [4p, 4p+4) (contiguous)
    xr = x.rearrange("b c (hp hs) w -> b c hp (hs w)", hp=P)
    yr = y.rearrange("b c (hp hs) w -> b c hp (hs w)", hp=P)

    f32 = mybir.dt.float32
    with tile.TileContext(nc) as tc, ExitStack() as ctx:
        in_pool = ctx.enter_context(tc.tile_pool(name="in", bufs=2))
        out_pool = ctx.enter_context(tc.tile_pool(name="out", bufs=2))
        t_pool = ctx.enter_context(tc.tile_pool(name="tmp", bufs=3))
        s_pool = ctx.enter_context(tc.tile_pool(name="scaled", bufs=6))

        for b in range(B_PER_CORE):
            it = in_pool.tile([P, C * F], f32)
            for c in range(C):
                nc.sync.dma_start(it[:, c * F:(c + 1) * F], xr[b, c])
            ot = out_pool.tile([P, C * F], f32)
            for c in range(C):
                # out_c = a[c,i]*X_i + a[c,j]*X_j + a[c,c]*X_c with the
                # diagonal term largest; (i, j) = off-diagonals, |a_i|<=|a_j|:
                #   s   = X_c * a[c,c]                      (ACT)
                #   t1  = X_i * (a[c,i]/a[c,j]) + X_j       (DVE stt)
                #   out = t1 * a[c,j] + s                   (DVE stt)
                i, j = [d for d in range(C) if d != c]
                if abs(a[c, i]) > abs(a[c, j]):
                    i, j = j, i
                sl = lambda d: slice(d * F, (d + 1) * F)
                s = s_pool.tile([P, F], f32)
                nc.scalar.mul(s[:], it[:, sl(c)], float(a[c, c]))
                t1 = t_pool.tile([P, F], f32)
                nc.vector.scalar_tensor_tensor(
                    t1[:], it[:, sl(i)], float(a[c, i] / a[c, j]), it[:, sl(j)],
                    mybir.AluOpType.mult, mybir.AluOpType.add,
                )
                nc.vector.scalar_tensor_tensor(
                    ot[:, sl(c)], t1[:], float(a[c, j]), s[:],
                    mybir.AluOpType.mult, mybir.AluOpType.add,
                )
            for c in range(C):
                nc.sync.dma_start(yr[b, c], ot[:, c * F:(c + 1) * F])

    nc.compile()
    return nc


_NC = None


def _get_nc():
    global _NC
    if _NC is None:
        _NC = build_nc()
    return _NC


def _in_maps(i_en):
    xs = np.ascontiguousarray(np.asarray(i_en, dtype=np.float32)).reshape(
        N_CORES, B_PER_CORE, C, H, W
    )
    return [{"x": xs[i]} for i in range(N_CORES)]


def kernel(i_co=None, i_en=None, **_):
    res = run_bass_kernel_spmd(_get_nc(), _in_maps(i_en), list(range(N_CORES)))
    return np.concatenate(
        [res.results[i]["y"] for i in range(N_CORES)], axis=0
    )
